# revision 29
# baseline (speedup 1.0000x reference)
"""Trainium2 Bass kernel: single-head causal attention, data-parallel over batch.

Per core (one batch element):
    Q = x @ w_q; K = x @ w_k; V = (x @ w_v1) @ w_v2
    out = softmax_causal(Q K^T / sqrt(64)) @ V

Sharding: batch 8 -> one element per NeuronCore, weights replicated.

Design notes:
- Host prep: x is transposed and cast to bf16 per shard (fed as x_t
  [E, S]); weights cast to bf16, pre-tiled for the lhsT layout, and the
  softmax scale is folded into w_q. All matmuls run bf16 with fp32 PSUM
  accumulation; output is fp32.
- Low-rank reassociation: V = Vp @ w_v2 has rank <= 64, so
  attn @ V = (attn @ Vp) @ w_v2. The numerator GEMM contracts to width
  64 instead of 1024 - 16x fewer FLOPs than materializing V.
- Scores are computed transposed (S^T = K Q^T) so P^T = exp(S^T) lands in
  the exact lhsT layout the (P^T)^T @ [Vp|1] matmul needs - the attention
  matrix is never transposed on chip. The ones column appended to Vp
  makes row 64 of the numerator the softmax denominator for free.
- Softmax skips max-subtraction: |scores| is O(10) here, exp stays finite.
- The denominator row is reshaped [1,512] -> [4,128] by a tiny SBUF DMA,
  PE-transposed to [128,4], and the divide rides the output copy as a
  per-partition tensor_scalar multiply.
- Causality at tile granularity: strips overlapping the diagonal compute
  and consume only columns q >= strip start (lo-trim), and the diagonal
  128x128 block is masked with a precomputed triangular bf16 mask.
- Projection blocks (PE-dense) are interleaved with the attention groups
  (exp-latency-paced) so the TensorEngine's activity monitor keeps the
  clock at full rate.
"""

import os
import sys

import numpy as np

for _p in ("/opt/trn_rl_repo", "/root/.axon_site/_ro/trn_rl_repo"):
    if os.path.isdir(_p) and _p not in sys.path:
        sys.path.insert(0, _p)
os.environ.setdefault("MYCRO_LOCAL_CACHE", "1")

import ml_dtypes  # noqa: E402
import concourse.bass as bass  # noqa: E402
import concourse.mybir as mybir  # noqa: E402
import concourse.tile as tile  # noqa: E402
from concourse import bacc  # noqa: E402
from concourse import bass_utils  # noqa: E402
from concourse.masks import make_identity, make_upper_triangular  # noqa: E402

F32 = mybir.dt.float32
BF16 = mybir.dt.bfloat16

B, S, E, D = 8, 2048, 1024, 64
P = 128
NS = S // P       # 16 s/q tiles
NE = E // P       # 8 E-chunks (projection contraction)
QG = 512          # q-group width
NQG = S // QG     # 4 q-groups
GT = QG // P      # 4 q-tiles per group
SCALE = D ** -0.5
EXP_FN = mybir.ActivationFunctionType.Exp


def build_kernel(nc):
    x_t = nc.dram_tensor("x_t", (E, S), BF16, kind="ExternalInput").ap()
    # weights pre-tiled on host: w_*[p, c*D+d] = w[c*128+p, d]
    w_q = nc.dram_tensor("w_q", (P, NE * D), BF16, kind="ExternalInput").ap()
    w_k = nc.dram_tensor("w_k", (P, NE * D), BF16, kind="ExternalInput").ap()
    w_v1 = nc.dram_tensor("w_v1", (P, NE * D), BF16, kind="ExternalInput").ap()
    w_v2 = nc.dram_tensor("w_v2", (D, E), BF16, kind="ExternalInput").ap()
    out = nc.dram_tensor("out", (S, E), F32, kind="ExternalOutput").ap()

    with tile.TileContext(nc) as tc:
        _body(tc, nc, x_t, w_q, w_k, w_v1, w_v2, out)


def _body(tc, nc, x_t, w_q, w_k, w_v1, w_v2, out):
    from contextlib import ExitStack

    with ExitStack() as ctx:
        const = ctx.enter_context(tc.tile_pool(name="const", bufs=1))
        big = ctx.enter_context(tc.tile_pool(name="big", bufs=1))
        ptp = ctx.enter_context(tc.tile_pool(name="ptp", bufs=10))
        outp = ctx.enter_context(tc.tile_pool(name="outp", bufs=3))
        small = ctx.enter_context(tc.tile_pool(name="small", bufs=4))
        psA = ctx.enter_context(tc.tile_pool(name="psA", bufs=4, space="PSUM"))
        psT = ctx.enter_context(tc.tile_pool(name="psT", bufs=2, space="PSUM"))
        psN = ctx.enter_context(tc.tile_pool(name="psN", bufs=2, space="PSUM"))

        # ---- x^T loads, ascending; first chunk split fine for fast start ----
        xT = big.tile([P, NE, S], BF16, tag="xT")  # xT[p, c, s] = x[s, c*128+p]
        xtv = x_t.rearrange("(c p) s -> p c s", p=P)
        # first the small weight tiles (needed by the very first matmul),
        # then x^T column blocks, split across both HWDGE queues
        wq_sb = const.tile([P, NE, D], BF16, tag="wq")
        wk_sb = const.tile([P, NE, D], BF16, tag="wk")
        wv1_sb = const.tile([P, NE, D], BF16, tag="wv1")
        wv2_sb = const.tile([D, E], BF16, tag="wv2")
        nc.sync.dma_start(wq_sb[:, :, :], w_q.rearrange("p (c d) -> p c d", d=D))
        nc.scalar.dma_start(wk_sb[:, :, :], w_k.rearrange("p (c d) -> p c d", d=D))
        nc.gpsimd.dma_start(wv1_sb[:, :, :],
                            w_v1.rearrange("p (c d) -> p c d", d=D))
        nc.gpsimd.dma_start(wv2_sb[:, :], w_v2)
        hw_engs = (nc.sync, nc.scalar)
        for k in range(NE):  # ng=0 in eight single-ec pieces
            hw_engs[k % 2].dma_start(xT[:, k, 0:QG], xtv[:, k, 0:QG])
        for ng in range(1, NQG):
            for k in range(2):
                h = NE // 2
                hw_engs[k].dma_start(
                    xT[:, k * h:(k + 1) * h, ng * QG:(ng + 1) * QG],
                    xtv[:, k * h:(k + 1) * h, ng * QG:(ng + 1) * QG])

        ident = const.tile([D, D], BF16, tag="ident")
        make_identity(nc, ident[:, :])
        ident4 = const.tile([GT, GT], F32, tag="ident4")
        make_identity(nc, ident4[:, :])
        tri = const.tile([P, P], BF16, tag="tri")
        # tri[s, q] = 1 where s <= q else 0 (valid causal region, S^T layout)
        make_upper_triangular(nc, tri[:, :], val=1.0, diag=True)

        # Q^T and K^T are produced together by column-tiled matmuls into one
        # [128, n] PSUM tile (Q rows 0:64, K rows 64:128). K^T is re-based to
        # partition 0 by a small SBUF->SBUF DMA so the scores matmul sees
        # both operands at base 0.
        qkt_sb = big.tile([P, S], BF16, tag="qkt")
        kt_sb = big.tile([D, S], BF16, tag="kt")
        vpt_sb = big.tile([D, S], BF16, tag="vpt")
        # Vp tile-wise as [s, 64+1] (numerator lhsT); ones column -> denom row
        vp_sb = big.tile([P, NS, D + 1], BF16, tag="vp")
        nc.vector.memset(vp_sb[:, :, D], 1.0)

        def proj_pass(w_sb, dst, ng):
            ps = psA.tile([D, QG], F32, tag="psA")
            for ec in range(NE):
                nc.tensor.matmul(
                    ps[:, :],
                    w_sb[:, ec, :],
                    xT[:, ec, ng * QG:(ng + 1) * QG],
                    start=(ec == 0),
                    stop=(ec == NE - 1),
                )
            nc.scalar.copy(dst[:, ng * QG:(ng + 1) * QG], ps[:, :])

        def proj_pass_qk(ng):
            """Q^T and K^T together: two column-tiled matmuls run
            concurrently in array column groups 0:64 and 64:128."""
            sl = slice(ng * QG, (ng + 1) * QG)
            ps = psA.tile([P, QG], F32, tag="psA")
            for ec in range(NE):
                nc.tensor.matmul(
                    ps[0:D, :], wq_sb[:, ec, :], xT[:, ec, sl],
                    start=(ec == 0), stop=(ec == NE - 1),
                    tile_position=(0, 0), skip_group_check=True)
                nc.tensor.matmul(
                    ps[D:P, :], wk_sb[:, ec, :], xT[:, ec, sl],
                    start=(ec == 0), stop=(ec == NE - 1),
                    tile_position=(0, D), skip_group_check=True)
            nc.scalar.copy(qkt_sb[:, sl], ps[:, :])
            nc.sync.dma_start(kt_sb[:, sl], qkt_sb[D:P, sl])

        def vp_transp(ng):
            for st in range(ng * GT, (ng + 1) * GT):
                pst = psT.tile([P, D], BF16, tag="psT")
                nc.tensor.transpose(pst[:, :], vpt_sb[:, st * P:(st + 1) * P],
                                    ident[:, :])
                nc.vector.tensor_copy(vp_sb[:, st, 0:D], pst[:, :])

        def emit_proj(ng):
            """Q^T, K^T, Vp^T columns for one 512-wide block + Vp tiles."""
            proj_pass_qk(ng)
            proj_pass(wv1_sb, vpt_sb, ng)
            vp_transp(ng)

        def emit_strip(qg, j):
            """One score strip -> exp'd, masked P^T slice (bf16)."""
            dt_blk = j - qg * GT  # diagonal block index within group
            lo = dt_blk * P if 0 < dt_blk < GT else 0
            ps = psA.tile([P, QG], F32, tag="psA")
            pt = ptp.tile([P, QG], BF16, tag="pt")
            nc.tensor.matmul(
                ps[:, lo:QG],
                kt_sb[:, j * P:(j + 1) * P],
                qkt_sb[0:D, qg * QG + lo:(qg + 1) * QG],
                start=True,
                stop=True,
            )
            nc.scalar.activation(pt[:, lo:QG], ps[:, lo:QG], EXP_FN)
            if 0 <= dt_blk < GT:
                # mask the diagonal 128x128 block (cols < lo of this strip
                # are never read: numerator MMs are lo-trimmed)
                nc.gpsimd.tensor_mul(
                    pt[:, dt_blk * P:(dt_blk + 1) * P],
                    pt[:, dt_blk * P:(dt_blk + 1) * P],
                    tri[:, :],
                )
            return (j, pt[:, lo:QG], lo)

        def emit_epilogue(qg, psn):
            """Denominator row -> per-partition recip; numerator -> bf16."""
            d_sb = small.tile([D + 1, QG], F32, tag="dsb")
            nc.vector.tensor_copy(d_sb[D:D + 1, :], psn[D:D + 1, :])
            d4 = small.tile([GT, P], F32, tag="d4")
            nc.sync.dma_start(d4[:, :], d_sb[D:D + 1, :])
            ps4 = psT.tile([P, GT], F32, tag="psT")
            nc.tensor.transpose(ps4[:, :], d4[:, :], ident4[:, :])
            recip = small.tile([P, GT], F32, tag="recip")
            nc.vector.reciprocal(recip[:, :], ps4[:, :])
            num_sb = small.tile([D, QG], BF16, tag="numsb")
            nc.scalar.copy(num_sb[:, :], psn[0:D, :])
            return num_sb, recip

        def out_tile(qg, t, num_sb, recip, split=False):
            i = qg * GT + t  # global q-tile index
            o_t = outp.tile([P, E], F32, tag="o")
            for eh in range(2):
                pso = psA.tile([P, QG], F32, tag="psA")
                nc.tensor.matmul(pso[:, :],
                                 num_sb[:, t * P:(t + 1) * P],
                                 wv2_sb[:, eh * QG:(eh + 1) * QG],
                                 start=True, stop=True)
                if split and eh == 0:  # tail: halve latency across ACT+DVE
                    nc.scalar.activation(o_t[:, 0:QG], pso[:, :],
                                         mybir.ActivationFunctionType.Copy,
                                         scale=recip[:, t:t + 1])
                else:
                    nc.vector.tensor_scalar_mul(
                        o_t[:, eh * QG:(eh + 1) * QG], pso[:, :],
                        recip[:, t:t + 1])
            hw_engs[t % 2].dma_start(out[i * P:(i + 1) * P, :], o_t[:, :])

        # Software-pipelined schedule: strips for group g+1 are produced one
        # full period ahead, so the numerator matmuls of period g always read
        # exp'd data - TensorE never waits on ACT latency. Each period
        # interleaves: numerator MMs of g (primary), projection passes for
        # block g+1, out tiles of g-1, and score strip pairs of g+1.
        emit_proj(0)
        entries = [emit_strip(0, j) for j in range(GT)]
        nr = {}
        for g in range(NQG):
            n_st = (g + 1) * GT
            items = []
            if g + 1 < NQG:
                ng = g + 1
                items.append(lambda ng=ng: proj_pass_qk(ng))
            if g - 1 >= 0:
                pn, pr = nr[g - 1]
                items.append(lambda pn=pn, pr=pr, g2=g - 1:
                             out_tile(g2, 0, pn, pr))
            if g + 1 < NQG:
                ng = g + 1
                items.append(lambda ng=ng: proj_pass(wv1_sb, vpt_sb, ng))
                items.append(lambda ng=ng: vp_transp(ng))
            if g - 1 >= 0:
                pn, pr = nr[g - 1]
                for t in range(1, GT):
                    items.append(lambda t=t, pn=pn, pr=pr, g2=g - 1:
                                 out_tile(g2, t, pn, pr))
            next_entries = []
            if g + 1 < NQG:
                for j in range((g + 2) * GT):
                    items.append(
                        lambda j=j, g2=g + 1, acc=next_entries:
                        acc.append(emit_strip(g2, j)))
            psn = psN.tile([D + 1, QG], F32, tag="psn")
            ii = 0
            for (j, pt_ap, lo) in entries:
                nc.tensor.matmul(
                    psn[:, lo:QG], vp_sb[:, j, :], pt_ap,
                    start=(j == 0), stop=(j == n_st - 1))
                if ii < len(items):
                    items[ii]()
                    ii += 1
            while ii < len(items):
                items[ii]()
                ii += 1
            nr[g] = emit_epilogue(g, psn)
            entries = next_entries
        num_sb, recip = nr[NQG - 1]
        for t in range(GT):
            out_tile(NQG - 1, t, num_sb, recip, split=True)

_CACHE = {}


def _get_compiled():
    if "nc" not in _CACHE:
        nc = bacc.Bacc("TRN2", target_bir_lowering=False, debug=False,
                       enable_asserts=False, num_devices=B)
        build_kernel(nc)
        nc.compile()
        _CACHE["nc"] = nc
    return _CACHE["nc"]


def _prep_w(w):
    """[E, D] -> pre-tiled [128, NE*D] bf16 with w'[p, c*D+d] = w[c*128+p, d]."""
    w = np.asarray(w, dtype=np.float32)
    return np.ascontiguousarray(
        w.reshape(NE, P, D).transpose(1, 0, 2).reshape(P, NE * D)
        .astype(ml_dtypes.bfloat16))


def _run(inputs, trace=False, tmpdir=None):
    nc = _get_compiled()
    bf16 = ml_dtypes.bfloat16
    x = np.asarray(inputs["x"], dtype=np.float32)
    w = {
        "w_q": _prep_w(np.asarray(inputs["w_q"], dtype=np.float32) * SCALE),
        "w_k": _prep_w(inputs["w_k"]),
        "w_v1": _prep_w(inputs["w_v1"]),
        "w_v2": np.ascontiguousarray(
            np.asarray(inputs["w_v2"], dtype=np.float32).astype(bf16)),
    }
    in_maps = [
        dict(x_t=np.ascontiguousarray(x[i].T.astype(bf16)), **w)
        for i in range(B)
    ]
    res = bass_utils.run_bass_kernel_spmd(
        nc, in_maps, core_ids=list(range(B)), trace=trace, tmpdir=tmpdir,
    )
    outs = np.stack([np.asarray(res.results[i]["out"]) for i in range(B)])
    return outs.astype(np.float32), res


def kernel(**inputs) -> np.ndarray:
    outs, _ = _run(inputs, trace=False)
    return outs


# revision 30
# speedup vs baseline: 1.0203x; 1.0203x over previous
"""Trainium2 Bass kernel: single-head causal attention, data-parallel over batch.

Per core (one batch element):
    Q = x @ w_q; K = x @ w_k; V = (x @ w_v1) @ w_v2
    out = softmax_causal(Q K^T / sqrt(64)) @ V

Sharding: batch 8 -> one element per NeuronCore, weights replicated.

Design notes:
- Host prep: x is transposed and cast to bf16 per shard (fed as x_t
  [E, S]); weights cast to bf16, pre-tiled for the lhsT layout, and the
  softmax scale is folded into w_q. All matmuls run bf16 with fp32 PSUM
  accumulation; output is fp32.
- Low-rank reassociation: V = Vp @ w_v2 has rank <= 64, so
  attn @ V = (attn @ Vp) @ w_v2. The numerator GEMM contracts to width
  64 instead of 1024 - 16x fewer FLOPs than materializing V.
- Scores are computed transposed (S^T = K Q^T) so P^T = exp(S^T) lands in
  the exact lhsT layout the (P^T)^T @ [Vp|1] matmul needs - the attention
  matrix is never transposed on chip. The ones column appended to Vp
  makes row 64 of the numerator the softmax denominator for free.
- Softmax skips max-subtraction: |scores| is O(10) here, exp stays finite.
- The denominator row is reshaped [1,512] -> [4,128] by a tiny SBUF DMA,
  PE-transposed to [128,4], and the divide rides the output copy as a
  per-partition tensor_scalar multiply.
- Causality at tile granularity: strips overlapping the diagonal compute
  and consume only columns q >= strip start (lo-trim), and the diagonal
  128x128 block is masked with a precomputed triangular bf16 mask.
- Projection blocks (PE-dense) are interleaved with the attention groups
  (exp-latency-paced) so the TensorEngine's activity monitor keeps the
  clock at full rate.
"""

import os
import sys

import numpy as np

for _p in ("/opt/trn_rl_repo", "/root/.axon_site/_ro/trn_rl_repo"):
    if os.path.isdir(_p) and _p not in sys.path:
        sys.path.insert(0, _p)
os.environ.setdefault("MYCRO_LOCAL_CACHE", "1")

import ml_dtypes  # noqa: E402
import concourse.bass as bass  # noqa: E402
import concourse.mybir as mybir  # noqa: E402
import concourse.tile as tile  # noqa: E402
from concourse import bacc  # noqa: E402
from concourse import bass_utils  # noqa: E402
from concourse.masks import make_identity, make_upper_triangular  # noqa: E402

F32 = mybir.dt.float32
BF16 = mybir.dt.bfloat16

B, S, E, D = 8, 2048, 1024, 64
P = 128
NS = S // P       # 16 s/q tiles
NE = E // P       # 8 E-chunks (projection contraction)
QG = 512          # q-group width
NQG = S // QG     # 4 q-groups
GT = QG // P      # 4 q-tiles per group
SCALE = D ** -0.5
EXP_FN = mybir.ActivationFunctionType.Exp


def build_kernel(nc):
    x_t = nc.dram_tensor("x_t", (E, S), BF16, kind="ExternalInput").ap()
    # weights pre-tiled on host: w_*[p, c*D+d] = w[c*128+p, d]
    w_q = nc.dram_tensor("w_q", (P, NE * D), BF16, kind="ExternalInput").ap()
    w_k = nc.dram_tensor("w_k", (P, NE * D), BF16, kind="ExternalInput").ap()
    w_v1 = nc.dram_tensor("w_v1", (P, NE * D), BF16, kind="ExternalInput").ap()
    w_v2 = nc.dram_tensor("w_v2", (D, E), BF16, kind="ExternalInput").ap()
    out = nc.dram_tensor("out", (S, E), F32, kind="ExternalOutput").ap()

    with tile.TileContext(nc) as tc:
        _body(tc, nc, x_t, w_q, w_k, w_v1, w_v2, out)


def _body(tc, nc, x_t, w_q, w_k, w_v1, w_v2, out):
    from contextlib import ExitStack

    with ExitStack() as ctx:
        const = ctx.enter_context(tc.tile_pool(name="const", bufs=1))
        big = ctx.enter_context(tc.tile_pool(name="big", bufs=1))
        ptp = ctx.enter_context(tc.tile_pool(name="ptp", bufs=10))
        outp = ctx.enter_context(tc.tile_pool(name="outp", bufs=3))
        small = ctx.enter_context(tc.tile_pool(name="small", bufs=4))
        psA = ctx.enter_context(tc.tile_pool(name="psA", bufs=4, space="PSUM"))
        psT = ctx.enter_context(tc.tile_pool(name="psT", bufs=2, space="PSUM"))
        psN = ctx.enter_context(tc.tile_pool(name="psN", bufs=2, space="PSUM"))

        # ---- x^T loads, ascending; first chunk split fine for fast start ----
        xT = big.tile([P, NE, S], BF16, tag="xT")  # xT[p, c, s] = x[s, c*128+p]
        xtv = x_t.rearrange("(c p) s -> p c s", p=P)
        # first the small weight tiles (needed by the very first matmul),
        # then x^T column blocks, split across both HWDGE queues
        wq_sb = const.tile([P, NE, D], BF16, tag="wq")
        wk_sb = const.tile([P, NE, D], BF16, tag="wk")
        wv1_sb = const.tile([P, NE, D], BF16, tag="wv1")
        wv2_sb = const.tile([D, E], BF16, tag="wv2")
        nc.sync.dma_start(wq_sb[:, :, :], w_q.rearrange("p (c d) -> p c d", d=D))
        nc.scalar.dma_start(wk_sb[:, :, :], w_k.rearrange("p (c d) -> p c d", d=D))
        nc.gpsimd.dma_start(wv1_sb[:, :, :],
                            w_v1.rearrange("p (c d) -> p c d", d=D))
        nc.gpsimd.dma_start(wv2_sb[:, :], w_v2)
        hw_engs = (nc.sync, nc.scalar)
        for k in range(NE):  # ng=0 in eight single-ec pieces
            hw_engs[k % 2].dma_start(xT[:, k, 0:QG], xtv[:, k, 0:QG])
        for ng in range(1, NQG):
            for k in range(2):
                h = NE // 2
                hw_engs[k].dma_start(
                    xT[:, k * h:(k + 1) * h, ng * QG:(ng + 1) * QG],
                    xtv[:, k * h:(k + 1) * h, ng * QG:(ng + 1) * QG])

        ident = const.tile([D, D], BF16, tag="ident")
        make_identity(nc, ident[:, :])
        ident4 = const.tile([GT, GT], F32, tag="ident4")
        make_identity(nc, ident4[:, :])
        tri = const.tile([P, P], BF16, tag="tri")
        # tri[s, q] = 1 where s <= q else 0 (valid causal region, S^T layout)
        make_upper_triangular(nc, tri[:, :], val=1.0, diag=True)

        # Q^T and K^T are produced together by column-tiled matmuls into one
        # [128, n] PSUM tile (Q rows 0:64, K rows 64:128). K^T is re-based to
        # partition 0 by a small SBUF->SBUF DMA so the scores matmul sees
        # both operands at base 0.
        qkt_sb = big.tile([P, S], BF16, tag="qkt")
        kt_sb = big.tile([D, S], BF16, tag="kt")
        vpt_sb = big.tile([D, S], BF16, tag="vpt")
        # Vp tile-wise as [s, 64+1] (numerator lhsT); ones column -> denom row
        vp_sb = big.tile([P, NS, D + 1], BF16, tag="vp")
        nc.vector.memset(vp_sb[:, :, D], 1.0)

        def proj_pass(w_sb, dst, ng):
            ps = psA.tile([D, QG], F32, tag="psA")
            for ec in range(NE):
                nc.tensor.matmul(
                    ps[:, :],
                    w_sb[:, ec, :],
                    xT[:, ec, ng * QG:(ng + 1) * QG],
                    start=(ec == 0),
                    stop=(ec == NE - 1),
                )
            nc.scalar.copy(dst[:, ng * QG:(ng + 1) * QG], ps[:, :])

        def proj_pass_qk(ng):
            """Q^T and K^T together: two column-tiled matmuls run
            concurrently in array column groups 0:64 and 64:128."""
            sl = slice(ng * QG, (ng + 1) * QG)
            ps = psA.tile([P, QG], F32, tag="psA")
            for ec in range(NE):
                nc.tensor.matmul(
                    ps[0:D, :], wq_sb[:, ec, :], xT[:, ec, sl],
                    start=(ec == 0), stop=(ec == NE - 1),
                    tile_position=(0, 0), skip_group_check=True)
                nc.tensor.matmul(
                    ps[D:P, :], wk_sb[:, ec, :], xT[:, ec, sl],
                    start=(ec == 0), stop=(ec == NE - 1),
                    tile_position=(0, D), skip_group_check=True)
            nc.scalar.copy(qkt_sb[:, sl], ps[:, :])
            nc.gpsimd.dma_start(kt_sb[:, sl], qkt_sb[D:P, sl])

        def vp_transp(ng):
            for st in range(ng * GT, (ng + 1) * GT):
                pst = psT.tile([P, D], BF16, tag="psT")
                nc.tensor.transpose(pst[:, :], vpt_sb[:, st * P:(st + 1) * P],
                                    ident[:, :])
                nc.vector.tensor_copy(vp_sb[:, st, 0:D], pst[:, :])

        def emit_proj(ng):
            """Q^T, K^T, Vp^T columns for one 512-wide block + Vp tiles."""
            proj_pass_qk(ng)
            proj_pass(wv1_sb, vpt_sb, ng)
            vp_transp(ng)

        def emit_strip(qg, j):
            """One score strip -> exp'd, masked P^T slice (bf16)."""
            dt_blk = j - qg * GT  # diagonal block index within group
            lo = dt_blk * P if 0 < dt_blk < GT else 0
            ps = psA.tile([P, QG], F32, tag="psA")
            pt = ptp.tile([P, QG], BF16, tag="pt")
            nc.tensor.matmul(
                ps[:, lo:QG],
                kt_sb[:, j * P:(j + 1) * P],
                qkt_sb[0:D, qg * QG + lo:(qg + 1) * QG],
                start=True,
                stop=True,
            )
            nc.scalar.activation(pt[:, lo:QG], ps[:, lo:QG], EXP_FN)
            if 0 <= dt_blk < GT:
                # mask the diagonal 128x128 block (cols < lo of this strip
                # are never read: numerator MMs are lo-trimmed)
                nc.gpsimd.tensor_mul(
                    pt[:, dt_blk * P:(dt_blk + 1) * P],
                    pt[:, dt_blk * P:(dt_blk + 1) * P],
                    tri[:, :],
                )
            return (j, pt[:, lo:QG], lo)

        def emit_epilogue(qg, psn):
            """Denominator row -> per-partition recip; numerator -> bf16."""
            d_sb = small.tile([D + 1, QG], F32, tag="dsb")
            nc.vector.tensor_copy(d_sb[D:D + 1, :], psn[D:D + 1, :])
            d4 = small.tile([GT, P], F32, tag="d4")
            nc.sync.dma_start(d4[:, :], d_sb[D:D + 1, :])
            ps4 = psT.tile([P, GT], F32, tag="psT")
            nc.tensor.transpose(ps4[:, :], d4[:, :], ident4[:, :])
            recip = small.tile([P, GT], F32, tag="recip")
            nc.vector.reciprocal(recip[:, :], ps4[:, :])
            num_sb = small.tile([D, QG], BF16, tag="numsb")
            nc.scalar.copy(num_sb[:, :], psn[0:D, :])
            return num_sb, recip

        def out_tile(qg, t, num_sb, recip, split=False):
            i = qg * GT + t  # global q-tile index
            o_t = outp.tile([P, E], F32, tag="o")
            for eh in range(2):
                pso = psA.tile([P, QG], F32, tag="psA")
                nc.tensor.matmul(pso[:, :],
                                 num_sb[:, t * P:(t + 1) * P],
                                 wv2_sb[:, eh * QG:(eh + 1) * QG],
                                 start=True, stop=True)
                if split and eh == 0:  # tail: halve latency across ACT+DVE
                    nc.scalar.activation(o_t[:, 0:QG], pso[:, :],
                                         mybir.ActivationFunctionType.Copy,
                                         scale=recip[:, t:t + 1])
                else:
                    nc.vector.tensor_scalar_mul(
                        o_t[:, eh * QG:(eh + 1) * QG], pso[:, :],
                        recip[:, t:t + 1])
            hw_engs[t % 2].dma_start(out[i * P:(i + 1) * P, :], o_t[:, :])

        # Software-pipelined schedule: strips for group g+1 are produced one
        # full period ahead, so the numerator matmuls of period g always read
        # exp'd data - TensorE never waits on ACT latency. Each period
        # interleaves: numerator MMs of g (primary), projection passes for
        # block g+1, out tiles of g-1, and score strip pairs of g+1.
        emit_proj(0)
        entries = [emit_strip(0, j) for j in range(GT)]
        nr = {}
        for g in range(NQG):
            n_st = (g + 1) * GT
            items = []
            if g + 1 < NQG:
                ng = g + 1
                items.append(lambda ng=ng: proj_pass_qk(ng))
            if g - 1 >= 0:
                pn, pr = nr[g - 1]
                items.append(lambda pn=pn, pr=pr, g2=g - 1:
                             out_tile(g2, 0, pn, pr))
            if g + 1 < NQG:
                ng = g + 1
                items.append(lambda ng=ng: proj_pass(wv1_sb, vpt_sb, ng))
                items.append(lambda ng=ng: vp_transp(ng))
            if g - 1 >= 0:
                pn, pr = nr[g - 1]
                for t in range(1, GT):
                    items.append(lambda t=t, pn=pn, pr=pr, g2=g - 1:
                                 out_tile(g2, t, pn, pr))
            next_entries = []
            if g + 1 < NQG:
                for j in range((g + 2) * GT):
                    items.append(
                        lambda j=j, g2=g + 1, acc=next_entries:
                        acc.append(emit_strip(g2, j)))
            psn = psN.tile([D + 1, QG], F32, tag="psn")
            ii = 0
            for (j, pt_ap, lo) in entries:
                nc.tensor.matmul(
                    psn[:, lo:QG], vp_sb[:, j, :], pt_ap,
                    start=(j == 0), stop=(j == n_st - 1))
                if ii < len(items):
                    items[ii]()
                    ii += 1
            while ii < len(items):
                items[ii]()
                ii += 1
            nr[g] = emit_epilogue(g, psn)
            entries = next_entries
        num_sb, recip = nr[NQG - 1]
        for t in range(GT):
            out_tile(NQG - 1, t, num_sb, recip, split=True)

_CACHE = {}


def _get_compiled():
    if "nc" not in _CACHE:
        nc = bacc.Bacc("TRN2", target_bir_lowering=False, debug=False,
                       enable_asserts=False, num_devices=B)
        build_kernel(nc)
        nc.compile()
        _CACHE["nc"] = nc
    return _CACHE["nc"]


def _prep_w(w):
    """[E, D] -> pre-tiled [128, NE*D] bf16 with w'[p, c*D+d] = w[c*128+p, d]."""
    w = np.asarray(w, dtype=np.float32)
    return np.ascontiguousarray(
        w.reshape(NE, P, D).transpose(1, 0, 2).reshape(P, NE * D)
        .astype(ml_dtypes.bfloat16))


def _run(inputs, trace=False, tmpdir=None):
    nc = _get_compiled()
    bf16 = ml_dtypes.bfloat16
    x = np.asarray(inputs["x"], dtype=np.float32)
    w = {
        "w_q": _prep_w(np.asarray(inputs["w_q"], dtype=np.float32) * SCALE),
        "w_k": _prep_w(inputs["w_k"]),
        "w_v1": _prep_w(inputs["w_v1"]),
        "w_v2": np.ascontiguousarray(
            np.asarray(inputs["w_v2"], dtype=np.float32).astype(bf16)),
    }
    in_maps = [
        dict(x_t=np.ascontiguousarray(x[i].T.astype(bf16)), **w)
        for i in range(B)
    ]
    res = bass_utils.run_bass_kernel_spmd(
        nc, in_maps, core_ids=list(range(B)), trace=trace, tmpdir=tmpdir,
    )
    outs = np.stack([np.asarray(res.results[i]["out"]) for i in range(B)])
    return outs.astype(np.float32), res


def kernel(**inputs) -> np.ndarray:
    outs, _ = _run(inputs, trace=False)
    return outs


# revision 31
# speedup vs baseline: 1.0898x; 1.0681x over previous
"""Trainium2 Bass kernel: single-head causal attention, data-parallel over batch.

Per core (one batch element):
    Q = x @ w_q; K = x @ w_k; V = (x @ w_v1) @ w_v2
    out = softmax_causal(Q K^T / sqrt(64)) @ V

Sharding: batch 8 -> one element per NeuronCore, weights replicated.

Design notes:
- Host prep: x is transposed and cast to bf16 per shard (fed as x_t
  [E, S]); weights cast to bf16, pre-tiled for the lhsT layout, and the
  softmax scale is folded into w_q. All matmuls run bf16 with fp32 PSUM
  accumulation; output is fp32.
- Low-rank reassociation: V = Vp @ w_v2 has rank <= 64, so
  attn @ V = (attn @ Vp) @ w_v2. The numerator GEMM contracts to width
  64 instead of 1024 - 16x fewer FLOPs than materializing V.
- Scores are computed transposed (S^T = K Q^T) so P^T = exp(S^T) lands in
  the exact lhsT layout the (P^T)^T @ [Vp|1] matmul needs - the attention
  matrix is never transposed on chip. The ones column appended to Vp
  makes row 64 of the numerator the softmax denominator for free.
- Softmax skips max-subtraction: |scores| is O(10) here, exp stays finite.
- The denominator row is reshaped [1,512] -> [4,128] by a tiny SBUF DMA,
  PE-transposed to [128,4], and the divide rides the output copy as a
  per-partition tensor_scalar multiply.
- Causality at tile granularity: strips overlapping the diagonal compute
  and consume only columns q >= strip start (lo-trim), and the diagonal
  128x128 block is masked with a precomputed triangular bf16 mask.
- Projection blocks (PE-dense) are interleaved with the attention groups
  (exp-latency-paced) so the TensorEngine's activity monitor keeps the
  clock at full rate.
"""

import os
import sys

import numpy as np

for _p in ("/opt/trn_rl_repo", "/root/.axon_site/_ro/trn_rl_repo"):
    if os.path.isdir(_p) and _p not in sys.path:
        sys.path.insert(0, _p)
os.environ.setdefault("MYCRO_LOCAL_CACHE", "1")

import ml_dtypes  # noqa: E402
import concourse.bass as bass  # noqa: E402
import concourse.mybir as mybir  # noqa: E402
import concourse.tile as tile  # noqa: E402
from concourse import bacc  # noqa: E402
from concourse import bass_utils  # noqa: E402
from concourse.masks import make_identity, make_upper_triangular  # noqa: E402

F32 = mybir.dt.float32
BF16 = mybir.dt.bfloat16

B, S, E, D = 8, 2048, 1024, 64
P = 128
NS = S // P       # 16 s/q tiles
NE = E // P       # 8 E-chunks (projection contraction)
QG = 512          # q-group width
NQG = S // QG     # 4 q-groups
GT = QG // P      # 4 q-tiles per group
SCALE = D ** -0.5
EXP_FN = mybir.ActivationFunctionType.Exp


def build_kernel(nc):
    x_t = nc.dram_tensor("x_t", (E, S), BF16, kind="ExternalInput").ap()
    # weights pre-tiled on host: w_*[p, c*D+d] = w[c*128+p, d]
    w_q = nc.dram_tensor("w_q", (P, NE * D), BF16, kind="ExternalInput").ap()
    w_k = nc.dram_tensor("w_k", (P, NE * D), BF16, kind="ExternalInput").ap()
    w_v1 = nc.dram_tensor("w_v1", (P, NE * D), BF16, kind="ExternalInput").ap()
    w_v2 = nc.dram_tensor("w_v2", (D, E), BF16, kind="ExternalInput").ap()
    out = nc.dram_tensor("out", (S, E), F32, kind="ExternalOutput").ap()

    with tile.TileContext(nc) as tc:
        _body(tc, nc, x_t, w_q, w_k, w_v1, w_v2, out)


def _body(tc, nc, x_t, w_q, w_k, w_v1, w_v2, out):
    from contextlib import ExitStack

    with ExitStack() as ctx:
        const = ctx.enter_context(tc.tile_pool(name="const", bufs=1))
        big = ctx.enter_context(tc.tile_pool(name="big", bufs=1))
        ptp = ctx.enter_context(tc.tile_pool(name="ptp", bufs=16))
        outp = ctx.enter_context(tc.tile_pool(name="outp", bufs=3))
        small = ctx.enter_context(tc.tile_pool(name="small", bufs=4))
        psA = ctx.enter_context(tc.tile_pool(name="psA", bufs=5, space="PSUM"))
        psT = ctx.enter_context(tc.tile_pool(name="psT", bufs=1, space="PSUM"))
        psN = ctx.enter_context(tc.tile_pool(name="psN", bufs=2, space="PSUM"))

        # ---- x^T loads, ascending; first chunk split fine for fast start ----
        xT = big.tile([P, NE, S], BF16, tag="xT")  # xT[p, c, s] = x[s, c*128+p]
        xtv = x_t.rearrange("(c p) s -> p c s", p=P)
        # first the small weight tiles (needed by the very first matmul),
        # then x^T column blocks, split across both HWDGE queues
        wq_sb = const.tile([P, NE, D], BF16, tag="wq")
        wk_sb = const.tile([P, NE, D], BF16, tag="wk")
        wv1_sb = const.tile([P, NE, D], BF16, tag="wv1")
        wv2_sb = const.tile([D, E], BF16, tag="wv2")
        nc.sync.dma_start(wq_sb[:, :, :], w_q.rearrange("p (c d) -> p c d", d=D))
        nc.scalar.dma_start(wk_sb[:, :, :], w_k.rearrange("p (c d) -> p c d", d=D))
        nc.gpsimd.dma_start(wv1_sb[:, :, :],
                            w_v1.rearrange("p (c d) -> p c d", d=D))
        nc.gpsimd.dma_start(wv2_sb[:, :], w_v2)
        hw_engs = (nc.sync, nc.scalar)
        for k in range(NE):  # ng=0 in eight single-ec pieces
            hw_engs[k % 2].dma_start(xT[:, k, 0:QG], xtv[:, k, 0:QG])
        for ng in range(1, NQG):
            for k in range(2):
                h = NE // 2
                hw_engs[k].dma_start(
                    xT[:, k * h:(k + 1) * h, ng * QG:(ng + 1) * QG],
                    xtv[:, k * h:(k + 1) * h, ng * QG:(ng + 1) * QG])

        ident = const.tile([D, D], BF16, tag="ident")
        make_identity(nc, ident[:, :])
        ident4 = const.tile([GT, GT], F32, tag="ident4")
        make_identity(nc, ident4[:, :])
        tri = const.tile([P, P], BF16, tag="tri")
        # tri[s, q] = 1 where s <= q else 0 (valid causal region, S^T layout)
        make_upper_triangular(nc, tri[:, :], val=1.0, diag=True)

        # Q^T and K^T are produced together by column-tiled matmuls into one
        # [128, n] PSUM tile (Q rows 0:64, K rows 64:128). K^T is re-based to
        # partition 0 by a small SBUF->SBUF DMA so the scores matmul sees
        # both operands at base 0.
        qkt_sb = big.tile([P, S], BF16, tag="qkt")
        kt_sb = big.tile([D, S], BF16, tag="kt")
        vpt_sb = big.tile([D, S], BF16, tag="vpt")
        # Vp tile-wise as [s, 64+1] (numerator lhsT); ones column -> denom row
        vp_sb = big.tile([P, NS, D + 1], BF16, tag="vp")
        nc.vector.memset(vp_sb[:, :, D], 1.0)

        def proj_pass(w_sb, dst, ng):
            ps = psA.tile([D, QG], F32, tag="psA")
            for ec in range(NE):
                nc.tensor.matmul(
                    ps[:, :],
                    w_sb[:, ec, :],
                    xT[:, ec, ng * QG:(ng + 1) * QG],
                    start=(ec == 0),
                    stop=(ec == NE - 1),
                )
            nc.scalar.copy(dst[:, ng * QG:(ng + 1) * QG], ps[:, :])

        def proj_pass_qk(ng):
            """Q^T and K^T together: two column-tiled matmuls run
            concurrently in array column groups 0:64 and 64:128."""
            sl = slice(ng * QG, (ng + 1) * QG)
            ps = psA.tile([P, QG], F32, tag="psA")
            for ec in range(NE):
                nc.tensor.matmul(
                    ps[0:D, :], wq_sb[:, ec, :], xT[:, ec, sl],
                    start=(ec == 0), stop=(ec == NE - 1),
                    tile_position=(0, 0), skip_group_check=True)
                nc.tensor.matmul(
                    ps[D:P, :], wk_sb[:, ec, :], xT[:, ec, sl],
                    start=(ec == 0), stop=(ec == NE - 1),
                    tile_position=(0, D), skip_group_check=True)
            nc.scalar.copy(qkt_sb[:, sl], ps[:, :])
            nc.gpsimd.dma_start(kt_sb[:, sl], qkt_sb[D:P, sl])

        def vp_transp(ng):
            for st in range(ng * GT, (ng + 1) * GT):
                pst = psT.tile([P, D], BF16, tag="psT")
                nc.tensor.transpose(pst[:, :], vpt_sb[:, st * P:(st + 1) * P],
                                    ident[:, :])
                nc.vector.tensor_copy(vp_sb[:, st, 0:D], pst[:, :])

        def emit_proj(ng):
            """Q^T, K^T, Vp^T columns for one 512-wide block + Vp tiles."""
            proj_pass_qk(ng)
            proj_pass(wv1_sb, vpt_sb, ng)
            vp_transp(ng)

        def emit_strip(qg, j):
            """One score strip -> exp'd, masked P^T slice (bf16)."""
            dt_blk = j - qg * GT  # diagonal block index within group
            lo = dt_blk * P if 0 < dt_blk < GT else 0
            ps = psA.tile([P, QG], F32, tag="psA")
            pt = ptp.tile([P, QG], BF16, tag="pt")
            nc.tensor.matmul(
                ps[:, lo:QG],
                kt_sb[:, j * P:(j + 1) * P],
                qkt_sb[0:D, qg * QG + lo:(qg + 1) * QG],
                start=True,
                stop=True,
            )
            nc.scalar.activation(pt[:, lo:QG], ps[:, lo:QG], EXP_FN)
            if 0 <= dt_blk < GT:
                # mask the diagonal 128x128 block (cols < lo of this strip
                # are never read: numerator MMs are lo-trimmed)
                nc.gpsimd.tensor_mul(
                    pt[:, dt_blk * P:(dt_blk + 1) * P],
                    pt[:, dt_blk * P:(dt_blk + 1) * P],
                    tri[:, :],
                )
            return (j, pt[:, lo:QG], lo)

        def emit_epilogue(qg, psn):
            """Denominator row -> per-partition recip; numerator -> bf16."""
            d_sb = small.tile([D + 1, QG], F32, tag="dsb")
            nc.vector.tensor_copy(d_sb[D:D + 1, :], psn[D:D + 1, :])
            d4 = small.tile([GT, P], F32, tag="d4")
            nc.sync.dma_start(d4[:, :], d_sb[D:D + 1, :])
            ps4 = psT.tile([P, GT], F32, tag="psT")
            nc.tensor.transpose(ps4[:, :], d4[:, :], ident4[:, :])
            recip = small.tile([P, GT], F32, tag="recip")
            nc.vector.reciprocal(recip[:, :], ps4[:, :])
            num_sb = small.tile([D, QG], BF16, tag="numsb")
            nc.scalar.copy(num_sb[:, :], psn[0:D, :])
            return num_sb, recip

        def out_tile(qg, t, num_sb, recip, split=False):
            i = qg * GT + t  # global q-tile index
            o_t = outp.tile([P, E], F32, tag="o")
            for eh in range(2):
                pso = psA.tile([P, QG], F32, tag="psA")
                nc.tensor.matmul(pso[:, :],
                                 num_sb[:, t * P:(t + 1) * P],
                                 wv2_sb[:, eh * QG:(eh + 1) * QG],
                                 start=True, stop=True)
                if split and eh == 0:  # tail: halve latency across ACT+DVE
                    nc.scalar.activation(o_t[:, 0:QG], pso[:, :],
                                         mybir.ActivationFunctionType.Copy,
                                         scale=recip[:, t:t + 1])
                else:
                    nc.vector.tensor_scalar_mul(
                        o_t[:, eh * QG:(eh + 1) * QG], pso[:, :],
                        recip[:, t:t + 1])
            hw_engs[t % 2].dma_start(out[i * P:(i + 1) * P, :], o_t[:, :])

        # Software-pipelined schedule: strips for group g+1 are produced one
        # full period ahead, so the numerator matmuls of period g always read
        # exp'd data - TensorE never waits on ACT latency. Each period
        # interleaves: numerator MMs of g (primary), projection passes for
        # block g+1, out tiles of g-1, and score strip pairs of g+1.
        emit_proj(0)
        entries = [emit_strip(0, j) for j in range(GT)]
        nr = {}
        for g in range(NQG):
            n_st = (g + 1) * GT
            items = []
            if g + 1 < NQG:
                ng = g + 1
                items.append(lambda ng=ng: proj_pass_qk(ng))
            if g - 1 >= 0:
                pn, pr = nr[g - 1]
                items.append(lambda pn=pn, pr=pr, g2=g - 1:
                             out_tile(g2, 0, pn, pr))
            if g + 1 < NQG:
                ng = g + 1
                items.append(lambda ng=ng: proj_pass(wv1_sb, vpt_sb, ng))
                items.append(lambda ng=ng: vp_transp(ng))
            if g - 1 >= 0:
                pn, pr = nr[g - 1]
                for t in range(1, GT):
                    items.append(lambda t=t, pn=pn, pr=pr, g2=g - 1:
                                 out_tile(g2, t, pn, pr))
            next_entries = []
            if g + 1 < NQG:
                for j in range((g + 2) * GT):
                    items.append(
                        lambda j=j, g2=g + 1, acc=next_entries:
                        acc.append(emit_strip(g2, j)))
            psn = psN.tile([D + 1, QG], F32, tag="psn")
            ii = 0
            for (j, pt_ap, lo) in entries:
                nc.tensor.matmul(
                    psn[:, lo:QG], vp_sb[:, j, :], pt_ap,
                    start=(j == 0), stop=(j == n_st - 1))
                if ii < len(items):
                    items[ii]()
                    ii += 1
            while ii < len(items):
                items[ii]()
                ii += 1
            nr[g] = emit_epilogue(g, psn)
            entries = next_entries
        num_sb, recip = nr[NQG - 1]
        for t in range(GT):
            out_tile(NQG - 1, t, num_sb, recip, split=True)

_CACHE = {}


def _get_compiled():
    if "nc" not in _CACHE:
        nc = bacc.Bacc("TRN2", target_bir_lowering=False, debug=False,
                       enable_asserts=False, num_devices=B)
        build_kernel(nc)
        nc.compile()
        _CACHE["nc"] = nc
    return _CACHE["nc"]


def _prep_w(w):
    """[E, D] -> pre-tiled [128, NE*D] bf16 with w'[p, c*D+d] = w[c*128+p, d]."""
    w = np.asarray(w, dtype=np.float32)
    return np.ascontiguousarray(
        w.reshape(NE, P, D).transpose(1, 0, 2).reshape(P, NE * D)
        .astype(ml_dtypes.bfloat16))


def _run(inputs, trace=False, tmpdir=None):
    nc = _get_compiled()
    bf16 = ml_dtypes.bfloat16
    x = np.asarray(inputs["x"], dtype=np.float32)
    w = {
        "w_q": _prep_w(np.asarray(inputs["w_q"], dtype=np.float32) * SCALE),
        "w_k": _prep_w(inputs["w_k"]),
        "w_v1": _prep_w(inputs["w_v1"]),
        "w_v2": np.ascontiguousarray(
            np.asarray(inputs["w_v2"], dtype=np.float32).astype(bf16)),
    }
    in_maps = [
        dict(x_t=np.ascontiguousarray(x[i].T.astype(bf16)), **w)
        for i in range(B)
    ]
    res = bass_utils.run_bass_kernel_spmd(
        nc, in_maps, core_ids=list(range(B)), trace=trace, tmpdir=tmpdir,
    )
    outs = np.stack([np.asarray(res.results[i]["out"]) for i in range(B)])
    return outs.astype(np.float32), res


def kernel(**inputs) -> np.ndarray:
    outs, _ = _run(inputs, trace=False)
    return outs
